# revision 25
# baseline (speedup 1.0000x reference)
"""NoisyDense forward for Trainium2, 8-core tensor-parallel.

out = relu(x @ (w_mu + w_sigma * outer(eps_in, eps_out)) + b_mu + b_sigma*eps_out)

Sharding: 2-way over batch x 4-way over units (8 cores).
Per core: x shard [2048, 4096] (batch rows), w shard [4096, 1024] (unit cols).

Strategy (v4):
  - Rank-1 noise path: for the NoisyDense init case (w_sigma rows identical),
    w_sigma * outer(eps_in, eps_out) is rank-1, so its output contribution is
    (x @ eps_in) * (sigma*eps_out)^T — a rank-1 update. The host ships
    v = x @ eps_in (0.01% of the kernel FLOPs) and the device applies the
    update in the epilogue. The 137 GFLOP x @ w_mu matmul stays on device,
    and W tiles feed the PE straight from DMA (no materialization pass).
  - x pre-transposed + pre-tiled on the HOST into [MP, ki, ko, m] bf16 so no
    on-chip transposes are needed.
  - bf16 operands end-to-end (fp32 PSUM accumulation). Tolerance is 2e-2;
    bf16 matmul error here is ~3e-3. Halves DMA vs fp32 and enables the
    compiler's fast-weight-load path (fp32 never gets FWL).
  - W resident in SBUF as two n-halves [128, 32, 512] bf16, streamed in
    4-ktile chunks. Warmup schedule: first WARM panels run n-half 0 only,
    then their n-half 1, so early compute needs only half of W.
  - Per (panel, half): 32 accumulating matmuls [128x128]@[128x512] into one
    PSUM bank; epilogue on DVE: psum + v[m]*sig[n] + b[n], then relu, cast
    bf16. Output bf16, host casts up to fp32.

Variants:
  - "rowsig": w_sigma rows all identical (true for NoisyDense init). Host
    ships sig_row = w_sigma[0,:]*eps_out and v = x_shard @ eps_in.
  - "general": arbitrary w_sigma. Host folds the noise into the shipped
    weight (w_mu + w_sigma*outer) and ships sig_row = v = 0; the same
    compiled kernel runs.
"""

import numpy as np

BATCH = 4096
IN_DIM = 4096
UNITS = 4096
MSHARDS = 2
NSHARDS = 4
MS = BATCH // MSHARDS      # 2048 rows of x per core
NS = UNITS // NSHARDS      # 1024 units per core
P = 128
KO = IN_DIM // P           # 32 k-tiles
MP = MS // P               # 16 m-panels per core
NFREE = 512                # matmul moving free dim (one PSUM bank of fp32)
NT = NS // NFREE           # 2 n-tiles per core
WCH = 4                    # w dma chunk size in k-tiles
WARM = min(4, MP)          # panels in the split-half warmup phase
XBUFS = WARM + 2           # x panel buffers (warmup panels stay resident)

_NC_CACHE = {}


def _build():
    from concourse import bacc
    import concourse.mybir as mybir
    import concourse.tile as tile

    f32 = mybir.dt.float32
    bf16 = mybir.dt.bfloat16
    mult = mybir.AluOpType.mult
    add = mybir.AluOpType.add

    nc = bacc.Bacc(None, target_bir_lowering=False, dynamic_dma_scratch_size=2048)

    xt_d = nc.dram_tensor("xt_s", [MP, P, KO, P], bf16, kind="ExternalInput")
    # W pre-tiled by n-half: [ki, half, ko, n-in-half]
    wmu_d = nc.dram_tensor("wmu_s", [P, NT, KO, NFREE], bf16, kind="ExternalInput")
    sig_d = nc.dram_tensor("sig_row", [NS], f32, kind="ExternalInput")
    b_d = nc.dram_tensor("b_row", [NS], f32, kind="ExternalInput")
    # v = x_shard @ eps_in, pre-arranged to [p, mp] (v_t[p, mp] = v[mp*P+p])
    v_d = nc.dram_tensor("v_t", [P, MP], f32, kind="ExternalInput")
    out_d = nc.dram_tensor("out_s", [MS, NS], bf16, kind="ExternalOutput")

    with tile.TileContext(nc) as tc:
        with (
            tc.tile_pool(name="const", bufs=1) as const,
            tc.tile_pool(name="wpool", bufs=1) as wpool,
            tc.tile_pool(name="xnat", bufs=XBUFS) as xnat,
            tc.tile_pool(name="outp", bufs=4) as outp,
            tc.tile_pool(name="cpool", bufs=4) as cpool,
            tc.tile_pool(name="ps", bufs=5, space="PSUM") as psp,
            tc.tile_pool(name="psh", bufs=2, space="PSUM") as psph,
        ):
            # ---- small constants ----
            v_sb = const.tile([P, MP], f32, tag="vt")
            nc.sync.dma_start(v_sb[:], v_d[:])

            # ---- x panel loads ----
            def issue_x(pm):
                xa = xnat.tile([P, KO, P], bf16, tag="xa")
                nc.sync.dma_start(xa[:], xt_d[pm])
                return xa

            pre_x = {0: issue_x(0)}

            # ---- W: two n-half tiles, usable chunk by chunk off DMA ----
            wh0 = wpool.tile([P, KO, NFREE], bf16, tag="wh0")
            wh1 = wpool.tile([P, KO, NFREE], bf16, tag="wh1")
            w_h = [wh0, wh1]

            def w_chunks(h):
                for c in range(KO // WCH):
                    ksl = slice(c * WCH, (c + 1) * WCH)
                    nc.sync.dma_start(w_h[h][:, ksl, :], wmu_d[:, h, ksl, :])

            # DMA issue order is the schedule: x0, wh0 (first MMs), x1,
            # epilogue consts, x2..x5, wh1, per-panel outs, then in-loop x.
            w_chunks(0)
            if 1 < MP:
                pre_x[1] = issue_x(1)
            sig_b = const.tile([P, NS], f32, tag="sigb")
            b_b = const.tile([P, NS], f32, tag="bb")
            with nc.allow_non_contiguous_dma(reason="one-time row broadcasts"):
                nc.sync.dma_start(sig_b[:], sig_d[None, :].to_broadcast([P, NS]))
                nc.sync.dma_start(b_b[:], b_d[None, :].to_broadcast([P, NS]))
            for pm in range(2, min(XBUFS, MP)):
                pre_x[pm] = issue_x(pm)
            w_chunks(1)

            # ---- compute one (panel, n-half) accumulation group ----
            def do_tile(xa, mp, nt):
                nsl = slice(nt * NFREE, (nt + 1) * NFREE)
                # rank-1 bias plane c = sig[n]*v[m] + b[n]: SBUF-only DVE op,
                # runs during the matmul group (only depends on constants)
                ct = cpool.tile([P, NFREE], f32, tag="ct")
                nc.vector.scalar_tensor_tensor(
                    out=ct[:],
                    in0=sig_b[:, nsl],
                    scalar=v_sb[:, mp : mp + 1],
                    in1=b_b[:, nsl],
                    op0=mult,
                    op1=add,
                )
                ps = psp.tile([P, NFREE], f32, tag="ps")
                for ko in range(KO):
                    nc.tensor.matmul(
                        ps[:],
                        xa[:, ko, :],
                        w_h[nt][:, ko, :],
                        start=(ko == 0),
                        stop=(ko == KO - 1),
                    )
                ot = outp.tile([P, NFREE], bf16, tag="ot")
                nc.vector.tensor_add(ot[:], ps[:], ct[:])
                nc.vector.tensor_scalar_max(ot[:], ot[:], 0.0)
                nc.sync.dma_start(out_d[mp * P : (mp + 1) * P, nsl], ot[:])

            def get_x(mp):
                return pre_x.pop(mp) if mp in pre_x else issue_x(mp)

            def ensure_x(mp):
                if mp < MP and mp not in pre_x:
                    pre_x[mp] = issue_x(mp)

            # warmup: first WARM panels, n-half 0 only (xa stays resident)
            warm_x = {}
            for mp in range(WARM):
                warm_x[mp] = get_x(mp)
                ensure_x(mp + XBUFS - 2)
                do_tile(warm_x[mp], mp, 0)
            # their n-half 1
            for mp in range(WARM):
                do_tile(warm_x.pop(mp), mp, 1)
            def do_tile_split(xa, mp, nt):
                # last group split into two 256-col halves in separate PSUM
                # banks: first half's epilogue overlaps second half's MMs,
                # trimming the end-of-kernel drain tail
                for half in range(2):
                    nsl = slice(nt * NFREE + half * 256, nt * NFREE + (half + 1) * 256)
                    ct = cpool.tile([P, 256], f32, tag="cth")
                    nc.vector.scalar_tensor_tensor(
                        out=ct[:],
                        in0=sig_b[:, nsl],
                        scalar=v_sb[:, mp : mp + 1],
                        in1=b_b[:, nsl],
                        op0=mult,
                        op1=add,
                    )
                    ps = psph.tile([P, 256], f32, tag="psh")
                    for ko in range(KO):
                        nc.tensor.matmul(
                            ps[:],
                            xa[:, ko, :],
                            w_h[nt][:, ko, nsl.start - nt * NFREE : nsl.stop - nt * NFREE],
                            start=(ko == 0),
                            stop=(ko == KO - 1),
                        )
                    ot = outp.tile([P, 256], bf16, tag="oth")
                    nc.vector.tensor_add(ot[:], ps[:], ct[:])
                    nc.vector.tensor_scalar_max(ot[:], ot[:], 0.0)
                    nc.sync.dma_start(out_d[mp * P : (mp + 1) * P, nsl], ot[:])

            # steady state
            for mp in range(WARM, MP):
                xa = get_x(mp)
                ensure_x(mp + XBUFS - 2)
                do_tile(xa, mp, 0)
                if mp == MP - 1:
                    do_tile_split(xa, mp, 1)
                else:
                    do_tile(xa, mp, 1)

    nc.compile()
    return nc


def get_nc(variant="rowsig"):
    # one compiled graph serves both variants (host prep differs)
    if "nc" not in _NC_CACHE:
        _NC_CACHE["nc"] = _build()
    return _NC_CACHE["nc"]


def pick_variant(w_sigma):
    w_sigma = np.asarray(w_sigma)
    return "rowsig" if bool((w_sigma == w_sigma[0:1, :]).all()) else "general"


def _bf16():
    import ml_dtypes

    return ml_dtypes.bfloat16


def shard_inputs(x, w_mu, w_sigma, b_mu, b_sigma, eps_in, eps_out, variant="rowsig"):
    bf16 = _bf16()
    x = np.asarray(x, dtype=np.float32)
    w_mu = np.asarray(w_mu, dtype=np.float32)
    w_sigma = np.asarray(w_sigma, dtype=np.float32)
    b_mu = np.asarray(b_mu, dtype=np.float32)
    b_sigma = np.asarray(b_sigma, dtype=np.float32)
    eps_in = np.asarray(eps_in, dtype=np.float32)
    eps_out = np.asarray(eps_out, dtype=np.float32)

    if variant == "rowsig":
        w_eff = w_mu
        sig_row = w_sigma[0, :] * eps_out
        v_full = x @ eps_in  # [BATCH] fp32 — 0.01% of kernel FLOPs
    else:
        w_eff = w_mu + w_sigma * np.outer(eps_in, eps_out)
        sig_row = np.zeros(UNITS, dtype=np.float32)
        v_full = np.zeros(BATCH, dtype=np.float32)
    b_row = b_mu + b_sigma * eps_out

    in_maps = []
    for c in range(MSHARDS * NSHARDS):
        mr, ncol = divmod(c, NSHARDS)
        msl = slice(mr * MS, (mr + 1) * MS)
        nsl = slice(ncol * NS, (ncol + 1) * NS)
        # [MP, m, KO, ki] -> [MP, ki, KO, m]
        xt = (
            x[msl, :]
            .astype(bf16)
            .reshape(MP, P, KO, P)
            .transpose(0, 3, 2, 1)
        )
        # [KO, ki, NT, n] -> [ki, NT, KO, n]
        wt = (
            w_eff[:, nsl]
            .astype(bf16)
            .reshape(KO, P, NT, NFREE)
            .transpose(1, 2, 0, 3)
        )
        m = {
            "xt_s": np.ascontiguousarray(xt),
            "wmu_s": np.ascontiguousarray(wt),
            "sig_row": np.ascontiguousarray(sig_row[nsl]),
            "b_row": np.ascontiguousarray(b_row[nsl]),
            "v_t": np.ascontiguousarray(v_full[msl].reshape(MP, P).T),
        }
        in_maps.append(m)
    return in_maps


def unshard_output(results):
    out = np.empty((BATCH, UNITS), dtype=np.float32)
    for c, rmap in enumerate(results):
        mr, ncol = divmod(c, NSHARDS)
        out[mr * MS : (mr + 1) * MS, ncol * NS : (ncol + 1) * NS] = np.asarray(
            rmap["out_s"]
        ).astype(np.float32)
    return out


def kernel(x, w_mu, w_sigma, b_mu, b_sigma, eps_in, eps_out):
    from concourse.bass_utils import run_bass_kernel_spmd

    variant = pick_variant(w_sigma)
    nc = get_nc(variant)
    in_maps = shard_inputs(
        x, w_mu, w_sigma, b_mu, b_sigma, eps_in, eps_out, variant=variant
    )
    res = run_bass_kernel_spmd(nc, in_maps, core_ids=list(range(8)))
    return unshard_output(res.results)


# revision 33
# speedup vs baseline: 1.3889x; 1.3889x over previous
"""NoisyDense forward for Trainium2, 8-core tensor-parallel.

out = relu(x @ (w_mu + w_sigma * outer(eps_in, eps_out)) + b_mu + b_sigma*eps_out)

Sharding: 2-way over batch x 4-way over units (8 cores).
Per core: x shard [2048, 4096] (batch rows), w shard [4096, 1024] (unit cols).

Strategy:
  - Rank-1 noise path: for the NoisyDense init case (w_sigma rows identical),
    w_sigma * outer(eps_in, eps_out) is rank-1, so its output contribution is
    (x @ eps_in) * (sigma*eps_out)^T — a rank-1 update. The host ships
    v = x @ eps_in (0.01% of the kernel FLOPs) and the device applies the
    update in the epilogue. The 137 GFLOP x @ w_mu matmul stays on device,
    and W tiles feed the PE straight from DMA (no materialization pass).
  - x pre-transposed + pre-tiled on the HOST into [MP, ki, ko, m] bf16 so no
    on-chip transposes are needed.
  - bf16 operands end-to-end (fp32 PSUM accumulation). Tolerance is 2e-2;
    bf16 matmul error here is ~2.4e-3. Halves DMA vs fp32 and enables the
    compiler's fast-weight-load path (fp32 never gets FWL).
  - W resident in SBUF as two n-halves [128, 32, 512] bf16, streamed in
    4-ktile chunks. Warmup schedule: first WARM panels run n-half 0 only,
    then their n-half 1, so early compute needs only half of W. DMA issue
    order doubles as the transfer schedule (single HW-DGE queue).
  - Per (panel, half): 32 accumulating matmuls [128x128]@[128x512] into one
    PSUM bank. Epilogue: the rank-1 bias plane c = v[m]*sig[n] + b[n] is
    built on DVE during the matmul group (SBUF-only), so the post-matmul
    chain is just add + relu-cast-bf16. The very last group is split into
    two 256-col halves in separate PSUM banks so its first epilogue overlaps
    its second half's matmuls. Output bf16, host casts up to fp32.
  - Per-core timeline-sim 236.3us vs 218.5us matmul roofline (1024 MMs x
    512 cols @ 2.4GHz); the rest is DMA-bandwidth-bound warmup (~9us),
    fixed NEFF prologue/teardown (~5.6us), and the final drain (~2us).

Variants:
  - "rowsig": w_sigma rows all identical (true for NoisyDense init). Host
    ships sig_row = w_sigma[0,:]*eps_out and v = x_shard @ eps_in.
  - "general": arbitrary w_sigma. Host folds the noise into the shipped
    weight (w_mu + w_sigma*outer) and ships sig_row = v = 0; the same
    compiled kernel runs.
"""

import numpy as np

BATCH = 4096
IN_DIM = 4096
UNITS = 4096
MSHARDS = 2
NSHARDS = 4
MS = BATCH // MSHARDS      # 2048 rows of x per core
NS = UNITS // NSHARDS      # 1024 units per core
P = 128
KO = IN_DIM // P           # 32 k-tiles
MP = MS // P               # 16 m-panels per core
NFREE = 512                # matmul moving free dim (one PSUM bank of fp32)
NT = NS // NFREE           # 2 n-tiles per core
WCH = 4                    # w dma chunk size in k-tiles
WARM = min(4, MP)          # panels in the split-half warmup phase
XBUFS = WARM + 2           # x panel buffers (warmup panels stay resident)

_NC_CACHE = {}


def _build():
    from concourse import bacc
    import concourse.mybir as mybir
    import concourse.tile as tile

    f32 = mybir.dt.float32
    bf16 = mybir.dt.bfloat16
    mult = mybir.AluOpType.mult
    add = mybir.AluOpType.add

    nc = bacc.Bacc(None, target_bir_lowering=False, dynamic_dma_scratch_size=2048)

    xt_d = nc.dram_tensor("xt_s", [MP, P, KO, P], bf16, kind="ExternalInput")
    # W pre-tiled by n-half: [ki, half, ko, n-in-half]
    wmu_d = nc.dram_tensor("wmu_s", [P, NT, KO, NFREE], bf16, kind="ExternalInput")
    sig_d = nc.dram_tensor("sig_row", [NS], f32, kind="ExternalInput")
    b_d = nc.dram_tensor("b_row", [NS], f32, kind="ExternalInput")
    # v = x_shard @ eps_in, pre-arranged to [p, mp] (v_t[p, mp] = v[mp*P+p])
    v_d = nc.dram_tensor("v_t", [P, MP], f32, kind="ExternalInput")
    out_d = nc.dram_tensor("out_s", [MS, NS], bf16, kind="ExternalOutput")

    with tile.TileContext(nc) as tc:
        with (
            tc.tile_pool(name="const", bufs=1) as const,
            tc.tile_pool(name="wpool", bufs=1) as wpool,
            tc.tile_pool(name="xnat", bufs=XBUFS) as xnat,
            tc.tile_pool(name="outp", bufs=4) as outp,
            tc.tile_pool(name="cpool", bufs=4) as cpool,
            tc.tile_pool(name="ps", bufs=5, space="PSUM") as psp,
            tc.tile_pool(name="psh", bufs=2, space="PSUM") as psph,
        ):
            # ---- small constants ----
            v_sb = const.tile([P, MP], f32, tag="vt")
            nc.sync.dma_start(v_sb[:], v_d[:])

            # ---- x panel loads (two half-DMAs: a panel's first ko-half can
            # feed its group's first 16 matmuls while the second half lands,
            # which lets mp1 start right after wh0's last chunk) ----
            def issue_x(pm):
                xa = xnat.tile([P, KO, P], bf16, tag="xa")
                nc.sync.dma_start(xa[:, 0 : KO // 2, :], xt_d[pm, :, 0 : KO // 2, :])
                nc.sync.dma_start(xa[:, KO // 2 :, :], xt_d[pm, :, KO // 2 :, :])
                return xa

            pre_x = {0: issue_x(0)}

            # ---- W: two n-half tiles, usable chunk by chunk off DMA ----
            wh0 = wpool.tile([P, KO, NFREE], bf16, tag="wh0")
            wh1 = wpool.tile([P, KO, NFREE], bf16, tag="wh1")
            w_h = [wh0, wh1]

            def w_chunks(h):
                for c in range(KO // WCH):
                    ksl = slice(c * WCH, (c + 1) * WCH)
                    nc.sync.dma_start(w_h[h][:, ksl, :], wmu_d[:, h, ksl, :])

            # DMA issue order is the schedule (single HW-DGE queue): x0,
            # wh0 (first MMs), x1..x3 (pass-1 panels), epilogue consts, wh1
            # (must land before pass 2 — x4/x5 after it would stall pass 2),
            # then x4/x5, outs, in-loop x.
            w_chunks(0)
            for pm in range(1, min(WARM, MP)):
                pre_x[pm] = issue_x(pm)
            sig_b = const.tile([P, NS], f32, tag="sigb")
            b_b = const.tile([P, NS], f32, tag="bb")
            with nc.allow_non_contiguous_dma(reason="one-time row broadcasts"):
                nc.sync.dma_start(sig_b[:], sig_d[None, :].to_broadcast([P, NS]))
                nc.sync.dma_start(b_b[:], b_d[None, :].to_broadcast([P, NS]))
            w_chunks(1)
            for pm in range(WARM, min(XBUFS, MP)):
                pre_x[pm] = issue_x(pm)

            # ---- compute one (panel, n-half) accumulation group ----
            def do_tile(xa, mp, nt):
                nsl = slice(nt * NFREE, (nt + 1) * NFREE)
                # rank-1 bias plane c = sig[n]*v[m] + b[n]: SBUF-only DVE op,
                # runs during the matmul group (only depends on constants)
                ct = cpool.tile([P, NFREE], f32, tag="ct")
                nc.vector.scalar_tensor_tensor(
                    out=ct[:],
                    in0=sig_b[:, nsl],
                    scalar=v_sb[:, mp : mp + 1],
                    in1=b_b[:, nsl],
                    op0=mult,
                    op1=add,
                )
                ps = psp.tile([P, NFREE], f32, tag="ps")
                for ko in range(KO):
                    nc.tensor.matmul(
                        ps[:],
                        xa[:, ko, :],
                        w_h[nt][:, ko, :],
                        start=(ko == 0),
                        stop=(ko == KO - 1),
                    )
                ot = outp.tile([P, NFREE], bf16, tag="ot")
                nc.vector.tensor_add(ot[:], ps[:], ct[:])
                nc.vector.tensor_scalar_max(ot[:], ot[:], 0.0)
                nc.sync.dma_start(out_d[mp * P : (mp + 1) * P, nsl], ot[:])

            def get_x(mp):
                return pre_x.pop(mp) if mp in pre_x else issue_x(mp)

            def ensure_x(mp):
                if mp < MP and mp not in pre_x:
                    pre_x[mp] = issue_x(mp)

            # warmup: first WARM panels, n-half 0 only (xa stays resident)
            warm_x = {}
            for mp in range(WARM):
                warm_x[mp] = get_x(mp)
                ensure_x(mp + XBUFS - 2)
                do_tile(warm_x[mp], mp, 0)
            # their n-half 1
            for mp in range(WARM):
                do_tile(warm_x.pop(mp), mp, 1)
            def do_tile_split(xa, mp, nt):
                # last group split into two 256-col halves in separate PSUM
                # banks: first half's epilogue overlaps second half's MMs,
                # trimming the end-of-kernel drain tail
                for half in range(2):
                    nsl = slice(nt * NFREE + half * 256, nt * NFREE + (half + 1) * 256)
                    ct = cpool.tile([P, 256], f32, tag="cth")
                    nc.vector.scalar_tensor_tensor(
                        out=ct[:],
                        in0=sig_b[:, nsl],
                        scalar=v_sb[:, mp : mp + 1],
                        in1=b_b[:, nsl],
                        op0=mult,
                        op1=add,
                    )
                    ps = psph.tile([P, 256], f32, tag="psh")
                    for ko in range(KO):
                        nc.tensor.matmul(
                            ps[:],
                            xa[:, ko, :],
                            w_h[nt][:, ko, nsl.start - nt * NFREE : nsl.stop - nt * NFREE],
                            start=(ko == 0),
                            stop=(ko == KO - 1),
                        )
                    ot = outp.tile([P, 256], bf16, tag="oth")
                    nc.vector.tensor_add(ot[:], ps[:], ct[:])
                    nc.vector.tensor_scalar_max(ot[:], ot[:], 0.0)
                    nc.sync.dma_start(out_d[mp * P : (mp + 1) * P, nsl], ot[:])

            # steady state
            for mp in range(WARM, MP):
                xa = get_x(mp)
                ensure_x(mp + XBUFS - 2)
                do_tile(xa, mp, 0)
                if mp == MP - 1:
                    do_tile_split(xa, mp, 1)
                else:
                    do_tile(xa, mp, 1)

    nc.compile()
    return nc


def get_nc(variant="rowsig"):
    # one compiled graph serves both variants (host prep differs)
    if "nc" not in _NC_CACHE:
        _NC_CACHE["nc"] = _build()
    return _NC_CACHE["nc"]


def pick_variant(w_sigma):
    w_sigma = np.asarray(w_sigma)
    return "rowsig" if bool((w_sigma == w_sigma[0:1, :]).all()) else "general"


def _bf16():
    import ml_dtypes

    return ml_dtypes.bfloat16


def shard_inputs(x, w_mu, w_sigma, b_mu, b_sigma, eps_in, eps_out, variant="rowsig"):
    bf16 = _bf16()
    x = np.asarray(x, dtype=np.float32)
    w_mu = np.asarray(w_mu, dtype=np.float32)
    w_sigma = np.asarray(w_sigma, dtype=np.float32)
    b_mu = np.asarray(b_mu, dtype=np.float32)
    b_sigma = np.asarray(b_sigma, dtype=np.float32)
    eps_in = np.asarray(eps_in, dtype=np.float32)
    eps_out = np.asarray(eps_out, dtype=np.float32)

    if variant == "rowsig":
        w_eff = w_mu
        sig_row = w_sigma[0, :] * eps_out
        v_full = x @ eps_in  # [BATCH] fp32 — 0.01% of kernel FLOPs
    else:
        w_eff = w_mu + w_sigma * np.outer(eps_in, eps_out)
        sig_row = np.zeros(UNITS, dtype=np.float32)
        v_full = np.zeros(BATCH, dtype=np.float32)
    b_row = b_mu + b_sigma * eps_out

    in_maps = []
    for c in range(MSHARDS * NSHARDS):
        mr, ncol = divmod(c, NSHARDS)
        msl = slice(mr * MS, (mr + 1) * MS)
        nsl = slice(ncol * NS, (ncol + 1) * NS)
        # [MP, m, KO, ki] -> [MP, ki, KO, m]
        xt = (
            x[msl, :]
            .astype(bf16)
            .reshape(MP, P, KO, P)
            .transpose(0, 3, 2, 1)
        )
        # [KO, ki, NT, n] -> [ki, NT, KO, n]
        wt = (
            w_eff[:, nsl]
            .astype(bf16)
            .reshape(KO, P, NT, NFREE)
            .transpose(1, 2, 0, 3)
        )
        m = {
            "xt_s": np.ascontiguousarray(xt),
            "wmu_s": np.ascontiguousarray(wt),
            "sig_row": np.ascontiguousarray(sig_row[nsl]),
            "b_row": np.ascontiguousarray(b_row[nsl]),
            "v_t": np.ascontiguousarray(v_full[msl].reshape(MP, P).T),
        }
        in_maps.append(m)
    return in_maps


def unshard_output(results):
    out = np.empty((BATCH, UNITS), dtype=np.float32)
    for c, rmap in enumerate(results):
        mr, ncol = divmod(c, NSHARDS)
        out[mr * MS : (mr + 1) * MS, ncol * NS : (ncol + 1) * NS] = np.asarray(
            rmap["out_s"]
        ).astype(np.float32)
    return out


def kernel(x, w_mu, w_sigma, b_mu, b_sigma, eps_in, eps_out):
    from concourse.bass_utils import run_bass_kernel_spmd

    variant = pick_variant(w_sigma)
    nc = get_nc(variant)
    in_maps = shard_inputs(
        x, w_mu, w_sigma, b_mu, b_sigma, eps_in, eps_out, variant=variant
    )
    res = run_bass_kernel_spmd(nc, in_maps, core_ids=list(range(8)))
    return unshard_output(res.results)
